# revision 68
# baseline (speedup 1.0000x reference)
"""Cross-attention kernel for Trainium2, 8 NeuronCores.

Problem: B=2, T=S=2048, DM=1024, H=16, HD=64, partial RoPE on first 32 dims.
Sharding: batch (2-way) x head-group (4-way, 4 heads each) = 8 cores.
Each core computes its head-group's contribution to the full output
(out_partial = attn_heads @ Wo_rows); host sums 4 partials per batch.

v2 design (Act-engine-paced):
  - exp is the only Activation-engine work in steady state; it paces the
    attention phase (128 x [128,1024] exp tiles ~ 127us).
  - AV computed transposed: out[t_block=128, hd=64] accumulating over s
    chunks with ex (scores exp, bf16) as the matmul stationary. Halves PE
    rows vs the [65, t] orientation.
  - softmax sum Z via tiny ones-column matmuls (free size 1).
  - normalization: DVE tensor_scalar_mul with per-partition 1/Z.
  - att pairs transposed back to [hd, t] via PE transpose for the output
    projection (contraction over 128 = 2 heads x 64).
  - rope: stream_shuffle (DVE partition shuffle in 32-groups) instead of
    SBUF shift DMAs.
  - projections JIT-interleaved into the attention stream: only K/Q half-0
    run before head 0; V proj runs inside head 0's loop, K/Q half-1 inside
    head 1's loop, in <=4-matmul chunks to avoid starving the Act engine.
    PSUM: scores(4) + av(2) + z(1) banks + 1 bank time-shared by pv/proj/
    transpose pools.
  - phase 3: 4 big output DMAs, PSUM->SBUF copies split across Act+DVE.
"""

import numpy as np

B, T, S, DM = 2, 2048, 2048, 1024
H, HD, N_ELEM = 16, 64, 32
HG = 4          # heads per core
NCORES = 8

_cached = {}


def _build_program():
    import concourse.bass as bass
    import concourse.tile as tile
    from concourse import bacc, mybir
    from concourse.bass import ts, ds

    f32 = mybir.dt.float32
    bf16 = mybir.dt.bfloat16
    Exp = mybir.ActivationFunctionType.Exp

    nc = bacc.Bacc(
        "TRN2",
        target_bir_lowering=False,
        debug=False,
        enable_asserts=False,
        num_devices=NCORES,
    )

    xT_d = nc.dram_tensor("xT", [DM, T], bf16, kind="ExternalInput").ap()
    yT_d = nc.dram_tensor("yT", [DM, S], bf16, kind="ExternalInput").ap()
    wq_d = nc.dram_tensor("wq", [DM, 256], bf16, kind="ExternalInput").ap()
    wk_d = nc.dram_tensor("wk", [DM, 256], bf16, kind="ExternalInput").ap()
    wv_d = nc.dram_tensor("wv", [DM, 256], bf16, kind="ExternalInput").ap()
    wo_d = nc.dram_tensor("wo", [256, DM], bf16, kind="ExternalInput").ap()
    cext_d = nc.dram_tensor("cext", [128, T], bf16, kind="ExternalInput").ap()
    sext_d = nc.dram_tensor("sext", [128, T], bf16, kind="ExternalInput").ap()
    out_d = nc.dram_tensor("out", [T, DM], bf16, kind="ExternalOutput").ap()

    SHIFT16 = [(i + 16) % 32 for i in range(32)]

    with tile.TileContext(nc) as tc:
        with tc.tile_pool(name="const", bufs=1) as const:
            # ---- load inputs, ordered for earliest first score ----
            wk_sb = const.tile([128, 8, 256], bf16, tag="wk")
            nc.sync.dma_start(out=wk_sb, in_=wk_d.rearrange("(k p) n -> p k n", p=128))
            wq_sb = const.tile([128, 8, 256], bf16, tag="wq")
            nc.sync.dma_start(out=wq_sb, in_=wq_d.rearrange("(k p) n -> p k n", p=128))

            xT_sb = const.tile([128, 8, T], bf16, tag="xT")
            xT_r = xT_d.rearrange("(k p) t -> p k t", p=128)
            yT_sb = const.tile([128, 8, S], bf16, tag="yT")
            yT_r = yT_d.rearrange("(k p) t -> p k t", p=128)
            cext_sb = const.tile([128, T], bf16, tag="cext")
            sext_sb = const.tile([128, T], bf16, tag="sext")
            wv_sb = const.tile([128, 8, 256], bf16, tag="wv")
            wo_sb = const.tile([128, 2, DM], bf16, tag="wo")

            # wv early: the first V-proj chunk sits early in the PE
            # stream and must not stall on a late wv arrival
            nc.sync.dma_start(out=wv_sb,
                              in_=wv_d.rearrange("(k p) n -> p k n", p=128))
            # critical set for the first scores first; xT fully before
            # yT-ch1 (first exp needs all of qt; kt-ch1 only from st=8)
            c0 = ds(0, 1024)
            c1 = ds(1024, 1024)
            nc.sync.dma_start(out=yT_sb[:, :, c0], in_=yT_r[:, :, c0])
            nc.sync.dma_start(out=cext_sb[:, c0], in_=cext_d[:, c0])
            nc.sync.dma_start(out=sext_sb[:, c0], in_=sext_d[:, c0])
            nc.sync.dma_start(out=xT_sb[:, :, c0], in_=xT_r[:, :, c0])
            nc.sync.dma_start(out=xT_sb[:, :, c1], in_=xT_r[:, :, c1])
            nc.sync.dma_start(out=cext_sb[:, c1], in_=cext_d[:, c1])
            nc.sync.dma_start(out=sext_sb[:, c1], in_=sext_d[:, c1])
            nc.sync.dma_start(out=yT_sb[:, :, c1], in_=yT_r[:, :, c1])
            # identity matrix for PE transpose: ident[p, f] = (f == p)
            colv = const.tile([128, 128], f32, tag="colv")
            nc.gpsimd.iota(colv, pattern=[[1, 128]], base=0, channel_multiplier=0,
                           allow_small_or_imprecise_dtypes=True)
            rowv = const.tile([128, 1], f32, tag="rowv")
            nc.gpsimd.iota(rowv, pattern=[[0, 1]], base=0, channel_multiplier=1,
                           allow_small_or_imprecise_dtypes=True)
            ident = const.tile([128, 128], bf16, tag="ident")
            nc.vector.tensor_scalar(
                ident, colv, rowv, None, mybir.AluOpType.is_equal
            )
            ones_sb = const.tile([128, 1], bf16, tag="ones")
            nc.vector.memset(ones_sb, 1.0)
            # preload the exp table at t~0 so LoadActFuncSet is off the
            # critical path (it otherwise runs right before the first real exp)
            dummy = const.tile([1, 1], f32, tag="dummy")
            nc.vector.memset(dummy, 0.0)
            nc.scalar.activation(dummy, dummy, Exp, scale=1.0)

            # persistent SBUF tensors
            kt = [const.tile([128, S], bf16, tag=f"kt{i}", name=f"kt{i}") for i in range(2)]
            qt = [const.tile([128, T], bf16, tag=f"qt{i}", name=f"qt{i}") for i in range(2)]
            vsb = const.tile([128, 16, HG, 64], bf16, tag="vsb")
            # att (post-transpose, [hd-pair, t]) in 4 t-groups per pair for
            # fine-grained phase-3 deps
            attp = [[const.tile([128, 512], bf16, tag=f"attp{i}_{g}",
                                name=f"attp{i}_{g}") for g in range(4)]
                    for i in range(2)]

            with tc.tile_pool(name="ropetmp", bufs=2) as rtp:

                def rope_start(dst, mt, tsl_base, width, ps):
                    """dst[:, tsl] = ps*cext; t2 = shuffle16(ps)*sext on
                    GPSIMD (otherwise idle)."""
                    tsl = ds(tsl_base, width)
                    shf = rtp.tile([128, width], f32, tag=f"shf{width}",
                                   name=f"shf_{dst[0].name}_{mt}_{tsl_base}")
                    nc.vector.stream_shuffle(shf, ps, SHIFT16)
                    nc.vector.tensor_mul(dst[mt][:, tsl], ps, cext_sb[:, tsl])
                    t2 = rtp.tile([128, width], bf16, tag=f"t2{width}",
                                  name=f"t2_{dst[0].name}_{mt}_{tsl_base}")
                    nc.gpsimd.tensor_mul(t2, shf, sext_sb[:, tsl])
                    return (dst, mt, tsl, t2)

                def rope_end(rs):
                    dst, mt, tsl, t2 = rs
                    nc.vector.tensor_add(dst[mt][:, tsl], dst[mt][:, tsl], t2)

                def rope_fin(dst, mt, tsl_base, width, ps):
                    rope_end(rope_start(dst, mt, tsl_base, width, ps))

                # ---- prologue: K/Q half-0 projections (own psum pool) ----
                with tc.tile_pool(name="pp0", bufs=2, space="PSUM") as pp0:
                    # PE p-state warmup: dep-free matmuls during the input
                    # DMAs so the PE clock is at full speed for the real work
                    wmt = const.tile([128, 512], bf16, tag="wm")
                    nc.vector.memset(wmt, 0.0)
                    wps = pp0.tile([128, 512], f32, tag="warm")
                    for i in range(16):
                        nc.tensor.matmul(
                            wps, lhsT=wmt[:, 0:128], rhs=wmt,
                            start=True, stop=True,
                        )
                    # ordered so the first exp's chain is shortest:
                    # K-th0, Q-th0 (ropes overlapped), then Q-th1 (first
                    # st needs all of qt), then K-th1 (needed from st=8)
                    def proj0(w_sb, act_sb, dst, th):
                        ps = pp0.tile([128, 1024], f32, tag="proj",
                                      name=f"ps0_{dst[0].name}_{th}")
                        for c in range(2):
                            for kk in range(8):
                                nc.tensor.matmul(
                                    ps[:, ts(c, 512)],
                                    lhsT=w_sb[:, kk, ds(0, 128)],
                                    rhs=act_sb[:, kk,
                                               ds(th * 1024 + c * 512, 512)],
                                    start=(kk == 0),
                                    stop=(kk == 7),
                                )
                        return ps

                    # sequential per-tile ropes: the Pool-semaphore wait of
                    # each tile's add breaks the scheduler's same-sem wait
                    # merging, so kt0's rope overlaps Qp0's matmuls
                    rope_fin(kt, 0, 0, 1024, proj0(wk_sb, yT_sb, kt, 0))
                    rope_fin(qt, 0, 0, 1024, proj0(wq_sb, xT_sb, qt, 0))
                    rope_fin(qt, 0, 1024, 1024, proj0(wq_sb, xT_sb, qt, 1))
                    # K-th1 (s-cols 1024:2048, needed only from st=8) is
                    # deferred into head 0's slots so the first scores'
                    # merged waits cannot bind to its late yT-ch1 DMA

                # out-proj weights: emitted after the prologue matmuls
                # so they stay out of the PE wait-merge
                nc.sync.dma_start(out=wo_sb,
                                  in_=wo_d.rearrange("(i p) n -> p i n", p=128))

                # ---- attention (Act-paced; sequential heads) ----
                avp = tc.alloc_tile_pool(name="avp", bufs=1, space="PSUM")
                zp = tc.alloc_tile_pool(name="zp", bufs=1, space="PSUM")
                scr = tc.alloc_tile_pool(name="scr", bufs=1, space="PSUM")
                scp = tc.alloc_tile_pool(name="scp", bufs=2, space="PSUM")
                exl = tc.alloc_tile_pool(name="exl", bufs=3)
                asbp = tc.alloc_tile_pool(name="asb", bufs=2)
                rcp = tc.alloc_tile_pool(name="rcp", bufs=2)
                if True:

                    # V projection chunks (h0): 4+4 matmuls into 1-bank psum
                    pv_cur = [None]

                    def vproj_chunk(st, half):
                        if half == 0:
                            pv_cur[0] = scr.tile([128, 256], f32, tag="scr",
                                                 name=f"pv{st}")
                        pv = pv_cur[0]
                        for kk in range(4 * half, 4 * half + 4):
                            nc.tensor.matmul(
                                pv,
                                lhsT=yT_sb[:, kk, ds(st * 128, 128)],
                                rhs=wv_sb[:, kk, :],
                                start=(kk == 0),
                                stop=(kk == 7),
                            )
                        if half == 1:
                            nc.vector.tensor_copy(
                                vsb[:, st, :, :],
                                pv.rearrange("p (h d) -> p h d", h=HG),
                            )

                    def proj_chunks(w_sb, act_sb, dst, mt, th, width=256):
                        """One K/Q projection tile as width-wide chunks
                        (8 matmuls + rope each) through the 1-bank scratch
                        ring."""
                        out = []
                        for c in range(1024 // width):
                            def chunk(c=c):
                                base = th * 1024 + c * width
                                ps = scr.tile(
                                    [128, width], f32, tag="scr",
                                    name=f"psd_{dst[0].name}_{mt}_{th}_{c}")
                                for kk in range(8):
                                    nc.tensor.matmul(
                                        ps,
                                        lhsT=w_sb[:, kk, ds(mt * 128, 128)],
                                        rhs=act_sb[:, kk, ds(base, width)],
                                        start=(kk == 0),
                                        stop=(kk == 7),
                                    )
                                rope_fin(dst, mt, base, width, ps)
                            out.append(chunk)
                        return out

                    deferred = []

                    pair_sb = None
                    for h in range(HG):
                        hp, ro = h // 2, (h % 2) * 64
                        if h == 0:
                            qt1_burst = []
                            deferred = proj_chunks(wk_sb, yT_sb, kt, 0, 1)
                        if h == 1:
                            deferred = []
                            for th in range(2):
                                deferred += proj_chunks(wk_sb, yT_sb, kt, 1, th)
                            for th in range(2):
                                deferred += proj_chunks(wq_sb, xT_sb, qt, 1, th)
                        av = avp.tile([128, 16, 64], f32, tag="av", name=f"av{h}")
                        zt = zp.tile([128, 16], f32, tag="z", name=f"z{h}")

                        if h % 2 == 0:
                            pair_sb = asbp.tile([128, 16, 128], bf16, tag="pair",
                                                name=f"pair{hp}")

                        def issue_av(st_p, ex_p, tbs=range(16), h=h,
                                     av=av, zt=zt):
                            # a start=True matmul zeroes (pending-zero) its
                            # whole 2KB PSUM bank, so exactly the FIRST write
                            # into each bank carries start=True; sibling
                            # accumulators rely on the pending-zero reads
                            for tb in tbs:
                                nc.tensor.matmul(
                                    av[:, tb, :],
                                    lhsT=ex_p[:, ds(tb * 128, 128)],
                                    rhs=vsb[:, st_p, h, :],
                                    start=(st_p == 0 and tb % 8 == 0),
                                    stop=(st_p == 15),
                                    skip_group_check=True,
                                )
                                nc.tensor.matmul(
                                    zt[:, tb:tb + 1],
                                    lhsT=ex_p[:, ds(tb * 128, 128)],
                                    rhs=ones_sb,
                                    start=(st_p == 0 and tb == 0),
                                    stop=(st_p == 15),
                                    skip_group_check=True,
                                )

                        prev = None
                        pend_b = None
                        for st in range(16):
                            ex = exl.tile([128, 2048], bf16, tag="ex",
                                          name=f"ex{h}_{st}")
                            for th in range(2):
                                sc = scp.tile([128, 1024], f32, tag="sc")
                                for c in range(2):
                                    nc.tensor.matmul(
                                        sc[:, ts(c, 512)],
                                        lhsT=kt[hp][ro:ro + 64, ds(st * 128, 128)],
                                        rhs=qt[hp][ro:ro + 64,
                                                   ds(th * 1024 + c * 512, 512)],
                                        start=True,
                                        stop=True,
                                    )
                                nc.scalar.activation(
                                    ex[:, ds(th * 1024, 1024)], sc, Exp,
                                    scale=0.125,
                                )
                                # JIT-interleaved projection work
                                if h == 0:
                                    vproj_chunk(st, th)
                                    while qt1_burst:
                                        qt1_burst.pop(0)()
                                    if deferred and th == 1:
                                        deferred.pop(0)()
                                elif deferred and th == 1:
                                    # one chunk per st keeps PE per-slot work
                                    # at ~Act pace instead of front-loading
                                    deferred.pop(0)()
                            if prev is not None:
                                st_p, ex_p = prev
                                if st_p == 0:
                                    # stagger st0's bank-B AVs one slot so
                                    # the previous head's norm reads of that
                                    # bank can drain without stalling PE
                                    issue_av(0, ex_p, range(8))
                                    pend_b = ex_p
                                else:
                                    if pend_b is not None:
                                        issue_av(0, pend_b, range(8, 16))
                                        pend_b = None
                                    issue_av(st_p, ex_p)
                            prev = (st, ex)
                        if pend_b is not None:
                            issue_av(0, pend_b, range(8, 16))
                        issue_av(*prev)
                        while deferred:
                            deferred.pop(0)()
                        if h == 3:
                            scp.release()   # free 4 banks for out-proj psum

                        # normalize: att[t, hd] = av[t, hd] / Z[t]
                        rec = rcp.tile([128, 16], f32, tag="rec", name=f"rec{h}")
                        nc.vector.reciprocal(rec, zt)
                        if h < 3:
                            for tb in range(16):
                                nc.vector.tensor_scalar_mul(
                                    pair_sb[:, tb, ds(ro, 64)], av[:, tb, :],
                                    rec[:, tb:tb + 1],
                                )
                        # pair 0 complete: transpose [t,128]->[128,t]
                        if h == 1:
                            for tg in range(4):
                                for ti in range(4):
                                    tp_t = scr.tile([128, 128], bf16,
                                                    tag="scr")
                                    nc.tensor.transpose(
                                        tp_t, pair_sb[:, tg * 4 + ti, :],
                                        ident,
                                    )
                                    nc.vector.tensor_copy(
                                        attp[0][tg][:, ds(ti * 128, 128)],
                                        tp_t,
                                    )

                    # ---- fused tail for head 3: per t-group, pipeline
                    # norm -> transpose -> out-proj -> DMA ----
                    out_r = out_d.rearrange("(a p) n -> p a n", p=128)
                    pop = tc.alloc_tile_pool(name="pop", bufs=4, space="PSUM")
                    osb = tc.alloc_tile_pool(name="osb", bufs=2)
                    Copy = mybir.ActivationFunctionType.Copy
                    for tg in range(4):
                        for tb in range(tg * 4, tg * 4 + 4):
                            if tb % 2 == 0:
                                nc.vector.tensor_scalar_mul(
                                    pair_sb[:, tb, ds(64, 64)], av[:, tb, :],
                                    rec[:, tb:tb + 1],
                                )
                            else:
                                nc.scalar.activation(
                                    pair_sb[:, tb, ds(64, 64)], av[:, tb, :],
                                    Copy, scale=rec[:, tb:tb + 1],
                                )
                        for ti in range(4):
                            tp_t = scr.tile([128, 128], bf16, tag="scr")
                            nc.tensor.transpose(
                                tp_t, pair_sb[:, tg * 4 + ti, :], ident,
                            )
                            nc.vector.tensor_copy(
                                attp[1][tg][:, ds(ti * 128, 128)], tp_t,
                            )
                        obg = osb.tile([128, 4, 1024], bf16, tag="obg")
                        for t4 in range(4):
                            for nn in range(2):
                                po = pop.tile([128, 512], f32, tag="po")
                                nc.tensor.matmul(
                                    po,
                                    lhsT=attp[0][tg][:, ds(t4 * 128, 128)],
                                    rhs=wo_sb[:, 0, ts(nn, 512)],
                                    start=True,
                                    stop=False,
                                )
                                nc.tensor.matmul(
                                    po,
                                    lhsT=attp[1][tg][:, ds(t4 * 128, 128)],
                                    rhs=wo_sb[:, 1, ts(nn, 512)],
                                    start=False,
                                    stop=True,
                                )
                                if nn == 0:
                                    nc.vector.tensor_copy(
                                        obg[:, t4, ts(nn, 512)], po)
                                else:
                                    nc.scalar.copy(obg[:, t4, ts(nn, 512)], po)
                        if tg < 3:
                            nc.sync.dma_start(
                                out=out_r[:, ds(tg * 4, 4), :], in_=obg
                            )
                        else:
                            # split the last DMA so the final drain overlaps
                            # the copies of the trailing half-group
                            nc.sync.dma_start(
                                out=out_r[:, ds(tg * 4, 2), :],
                                in_=obg[:, 0:2, :],
                            )
                            nc.sync.dma_start(
                                out=out_r[:, ds(tg * 4 + 2, 2), :],
                                in_=obg[:, 2:4, :],
                            )
                    pop.release()
                    scr.release()
                    zp.release()
                    osb.release()
                    scp = None
                for p in (rcp, asbp, exl, avp):
                    p.release()

    nc.compile()
    return nc


def _rope_tables(cos=None, sin=None):
    """cext/sext [128, T] f32 for the [hd, t] layout (head pairs per tile).

    Rows r (rr = r % 64): rr<32 rope rows, else passthrough.
    cext: cos[t, rr%16] on rope rows, 1.0 on pass rows.
    sext is multiplied at the DEST row after the 16-shift (shf[r] =
    ps[(r+16)%32 within the 32-group]):
      rr<16: -sin[t, rr]; 16<=rr<32: +sin[t, rr-16]; else 0.
    """
    if cos is None or sin is None:
        inv_freq = 1.0 / (10000.0 ** (np.arange(0, N_ELEM, 2, dtype=np.float32) / N_ELEM))
        ang = np.arange(T, dtype=np.float32)[:, None] * inv_freq[None, :]
        cos, sin = np.cos(ang), np.sin(ang)
    cosT = np.ascontiguousarray(np.asarray(cos, dtype=np.float32).T)  # [16, T]
    sinT = np.ascontiguousarray(np.asarray(sin, dtype=np.float32).T)
    cext = np.ones((128, T), np.float32)
    sext = np.zeros((128, T), np.float32)
    for blk in (0, 64):
        for r in range(16):
            cext[blk + r] = cosT[r]
            cext[blk + 16 + r] = cosT[r]
            sext[blk + r] = -sinT[r]
            sext[blk + 16 + r] = sinT[r]
    return cext, sext


def _make_in_maps(x, y, Wq, Wk, Wv, Wo, cos=None, sin=None):
    import ml_dtypes

    bf = ml_dtypes.bfloat16
    cext, sext = _rope_tables(cos, sin)
    cext = cext.astype(bf)
    sext = sext.astype(bf)
    x = np.asarray(x, dtype=np.float32)
    y = np.asarray(y, dtype=np.float32)
    Wq = np.asarray(Wq, dtype=np.float32)
    Wk = np.asarray(Wk, dtype=np.float32)
    Wv = np.asarray(Wv, dtype=np.float32)
    Wo = np.asarray(Wo, dtype=np.float32)

    in_maps = []
    for c in range(NCORES):
        b, hg = c // 4, c % 4
        cs = slice(hg * 256, (hg + 1) * 256)
        in_maps.append({
            "xT": np.ascontiguousarray(x[b].T).astype(bf),
            "yT": np.ascontiguousarray(y[b].T).astype(bf),
            "wq": np.ascontiguousarray(Wq[:, cs]).astype(bf),
            "wk": np.ascontiguousarray(Wk[:, cs]).astype(bf),
            "wv": np.ascontiguousarray(Wv[:, cs]).astype(bf),
            "wo": np.ascontiguousarray(Wo[cs, :]).astype(bf),
            "cext": cext,
            "sext": sext,
        })
    return in_maps


def kernel(x, y, cos, sin, mask, Wq, Wk, Wv, Wo):
    from concourse.bass_utils import run_bass_kernel_spmd

    if "nc" not in _cached:
        _cached["nc"] = _build_program()
    nc = _cached["nc"]

    in_maps = _make_in_maps(x, y, Wq, Wk, Wv, Wo, cos, sin)
    res = run_bass_kernel_spmd(nc, in_maps, core_ids=list(range(NCORES)))
    parts = [np.asarray(r["out"], dtype=np.float32) for r in res.results]
    out = np.stack([
        parts[0] + parts[1] + parts[2] + parts[3],
        parts[4] + parts[5] + parts[6] + parts[7],
    ]).astype(np.float32)
    return out
